# revision 6
# baseline (speedup 1.0000x reference)
"""Trainium2 Bass kernel for nn_Attention_49598282334528.

Dense transformer attention block: fused QKV projection + RoPE + causal
GQA flash-style attention + output projection, for
  x: [2, 2048, 2048], H=16 q heads, KV=4 kv heads, head_dim=128.

Sharding (8 NeuronCores): data-parallel over batch (2) x tensor-parallel
over kv-head groups (4).  Core c handles batch c//4, kv-group c%4 (4 q
heads + 1 kv head).  Each core computes a full-width partial of the
output projection (row-parallel Wo); the host sums the 4 partials per
batch and stacks batches.

v2 design (software-pipelined, bf16 matmuls):
  - All matmul operands are bf16 (PE runs 1 cycle/row for bf16 at any
    output width, same as fp32r at >=256, but with no narrow-tile
    penalty); accumulation stays fp32 in PSUM.  Halves DMA and SBUF.
  - Single per-block pipeline: proj(b) -> attn(b) -> outproj(b), all
    pools open program-wide so the Tile scheduler can overlap phases.
  - Projection outputs (4 q heads, k, v) are computed sequentially,
    each evicted psum->sbuf(bf16) by the scalar engine; RoPE runs on
    the DVE in 4 ops/head on SBUF bf16 (2x DVE mode) using
    host-prepared [cos;cos] and [-sin;sin] tables.
  - Causal masking of diagonal score tiles via DVE multiply with a 0/1
    triangular tile (no PE mask matmuls); score/PV/rowsum matmuls are
    fully trimmed to the causal region.
  - Softmax denominators via an all-ones matmul accumulated alongside
    PV; normalization (reciprocal+mul) on DVE.
  - Output projection partials evicted psum->sbuf by DVE, DMA'd out
    per [128,512] tile.
  - DMAs are split per ~0.5MB chunk and emitted in need-order so the
    first matmul starts ~3us in instead of waiting for all weights.
"""

import sys

if "/opt/trn_rl_repo" not in sys.path:
    sys.path.insert(0, "/opt/trn_rl_repo")

import numpy as np

B, S, D = 2, 2048, 2048
H, KV, HD = 16, 4, 128
G = 4                # kv groups == cores per batch
QPH = H // KV        # q heads per group = 4
EQ = QPH * HD        # per-core q width = 512
NCORES = 8
P = 128
ABLK = 512           # seq block (both proj and attention sq block)
NA = S // ABLK       # 4
ND = D // P          # 16 contraction chunks
SCALE = 1.0 / float(np.sqrt(HD))

_CACHE = {}


def _build_program():
    import concourse.bass as bass
    import concourse.tile as tile
    from concourse import bacc, mybir

    f32 = mybir.dt.float32
    bf16 = mybir.dt.bfloat16
    EXP = mybir.ActivationFunctionType.Exp
    COPY = mybir.ActivationFunctionType.Copy

    nc = bacc.Bacc("TRN2", target_bir_lowering=False, debug=False)

    # host-prearranged inputs (see _prep_inputs for layouts)
    xtb = nc.dram_tensor("xtb", [NA, P, ND, ABLK], bf16, kind="ExternalInput").ap()
    wq = nc.dram_tensor("wq", [P, ND, EQ], bf16, kind="ExternalInput").ap()
    wk = nc.dram_tensor("wk", [P, ND, HD], bf16, kind="ExternalInput").ap()
    wv = nc.dram_tensor("wv", [P, ND, HD], bf16, kind="ExternalInput").ap()
    wo = nc.dram_tensor("wo", [P, QPH, D], bf16, kind="ExternalInput").ap()
    cc = nc.dram_tensor("cc", [P, S], bf16, kind="ExternalInput").ap()      # [cos;cos]
    sspm = nc.dram_tensor("sspm", [P, S], bf16, kind="ExternalInput").ap()  # [-sin;sin]
    ones_d = nc.dram_tensor("ones_d", [P, P], bf16, kind="ExternalInput").ap()
    ident_d = nc.dram_tensor("ident_d", [P, P], bf16, kind="ExternalInput").ap()
    mask01_d = nc.dram_tensor("mask01_d", [P, P], bf16, kind="ExternalInput").ap()
    outp = nc.dram_tensor("outp", [S, D], f32, kind="ExternalOutput").ap()

    with tile.TileContext(nc) as tc:
        import contextlib

        with contextlib.ExitStack() as stack:
            const = stack.enter_context(tc.tile_pool(name="const", bufs=1))
            persist = stack.enter_context(tc.tile_pool(name="persist", bufs=1))
            wpool = stack.enter_context(tc.tile_pool(name="wproj", bufs=1))
            xtp = stack.enter_context(tc.tile_pool(name="xtp", bufs=4))
            # psum pools: 2 + 3 + 2 + 1 = 8 banks
            pj = stack.enter_context(
                tc.tile_pool(name="pjps", bufs=2, space="PSUM"))
            stvt = stack.enter_context(
                tc.tile_pool(name="stps", bufs=3, space="PSUM"))
            ac = stack.enter_context(
                tc.tile_pool(name="acps", bufs=1, space="PSUM"))
            opl = stack.enter_context(
                tc.tile_pool(name="opps", bufs=1, space="PSUM"))
            # sbuf working pools
            pevt = stack.enter_context(tc.tile_pool(name="pevt", bufs=2))
            rtmp = stack.enter_context(tc.tile_pool(name="rtmp", bufs=2))
            stsb = stack.enter_context(tc.tile_pool(name="stsb", bufs=5))
            nrm = stack.enter_context(tc.tile_pool(name="nrm", bufs=2))
            osg = stack.enter_context(tc.tile_pool(name="osg", bufs=3))

            # ---- persistent tensors ----
            cc_sb = const.tile([P, S], bf16)
            ss_sb = const.tile([P, S], bf16)
            ones_sb = const.tile([P, P], bf16)
            ident_sb = const.tile([P, P], bf16)
            mask01_sb = const.tile([P, P], bf16)

            wq_sb = wpool.tile([P, ND, EQ], bf16)
            wk_sb = wpool.tile([P, ND, HD], bf16)
            wv_sb = wpool.tile([P, ND, HD], bf16)
            wo_sb = wpool.tile([P, QPH, D], bf16)

            qT_blks = [persist.tile([P, QPH, ABLK], bf16, name=f"qTb{b}")
                       for b in range(NA)]
            kT_blks = [persist.tile([P, ABLK], bf16, name=f"kTb{b}")
                       for b in range(NA)]
            v_blks = [persist.tile([P, ABLK // P, HD], bf16, name=f"vb{b}")
                      for b in range(NA)]
            oT_blks = [persist.tile([P, QPH, ABLK], bf16, name=f"oTb{b}")
                       for b in range(NA)]

            # ---- DMA emission, need-order ----
            # first: weights for q0 + x block 0, then consts, then the rest
            def dma(dst, src):
                nc.sync.dma_start(out=dst, in_=src)

            xt_tiles = [None] * NA

            def load_x_block(blk):
                t = xtp.tile([P, ND, ABLK], bf16, tag="xt", name=f"xt{blk}")
                for g in range(4):
                    dma(t[:, 4 * g : 4 * g + 4, :],
                        xtb[blk, :, 4 * g : 4 * g + 4, :])
                xt_tiles[blk] = t

            # q weights (first chunk first), x block 0
            dma(wq_sb[:, 0:4, :], wq[:, 0:4, :])
            load_x_block(0)
            for g in range(1, 4):
                dma(wq_sb[:, 4 * g : 4 * g + 4, :], wq[:, 4 * g : 4 * g + 4, :])
            dma(cc_sb[:], cc[:])
            dma(ss_sb[:], sspm[:])
            dma(ones_sb[:], ones_d[:])
            dma(ident_sb[:], ident_d[:])
            dma(mask01_sb[:], mask01_d[:])
            for g in range(2):
                dma(wk_sb[:, 8 * g : 8 * g + 8, :], wk[:, 8 * g : 8 * g + 8, :])
                dma(wv_sb[:, 8 * g : 8 * g + 8, :], wv[:, 8 * g : 8 * g + 8, :])
            load_x_block(1)
            for h in range(QPH):
                dma(wo_sb[:, h, :], wo[:, h, :])
            load_x_block(2)
            load_x_block(3)

            # ---- per-block pipeline ----
            def rope(pe_sb, s0, dst):
                """dst = rope(pe_sb) using [cos;cos] / [-sin;sin] tables."""
                HH = HD // 2
                cp = rtmp.tile([P, ABLK], bf16, tag="cp", name="cp")
                tm = rtmp.tile([P, ABLK], bf16, tag="tm", name="tm")
                nc.vector.tensor_mul(cp[:], pe_sb[:], cc_sb[:, s0 : s0 + ABLK])
                # ss_sb rows 64:128 hold -sin, rows 0:64 hold +sin, so each
                # mul's two SBUF inputs share a base partition (hw constraint)
                nc.vector.tensor_mul(
                    tm[0:HH, :], pe_sb[HH:P, :], ss_sb[HH:P, s0 : s0 + ABLK])
                nc.vector.tensor_mul(
                    tm[HH:P, :], pe_sb[0:HH, :], ss_sb[0:HH, s0 : s0 + ABLK])
                nc.vector.tensor_add(dst, cp[:], tm[:])

            def proj(blk):
                s0 = blk * ABLK
                xt_t = xt_tiles[blk]
                # outputs: q0..q3, k, v
                for oi in range(6):
                    pp = pj.tile([P, ABLK], f32, tag="pp", name="pp")
                    for di in range(ND):
                        if oi < QPH:
                            w = wq_sb[:, di, oi * HD : (oi + 1) * HD]
                        elif oi == QPH:
                            w = wk_sb[:, di, :]
                        else:
                            w = wv_sb[:, di, :]
                        nc.tensor.matmul(
                            pp[:], w, xt_t[:, di, :],
                            start=(di == 0), stop=(di == ND - 1),
                        )
                    pe = pevt.tile([P, ABLK], bf16, tag="pe", name="pe")
                    nc.scalar.activation(pe[:], pp[:], COPY)
                    if oi < QPH:
                        rope(pe, s0, qT_blks[blk][:, oi, :])
                    elif oi == QPH:
                        rope(pe, s0, kT_blks[blk][:])
                    else:
                        for j in range(ABLK // P):
                            vp = stvt.tile([P, P], bf16, tag="st", name="vtp")
                            nc.tensor.transpose(
                                vp[:], pe[:, j * P : (j + 1) * P], ident_sb[:])
                            nc.scalar.activation(
                                v_blks[blk][:, j, :], vp[:], COPY)

            def attn(blk):
                s0 = blk * ABLK
                n_sk = (blk + 1) * (ABLK // P)
                for h in range(QPH):
                    qT = qT_blks[blk][:, h, :]
                    oT_ps = ac.tile([P, ABLK], f32, tag="oT", name="oTps")
                    sm_ps = ac.tile([P, ABLK], f32, tag="sm", name="smps")

                    st_ps_l = [None] * n_sk
                    st_t_l = [None] * n_sk

                    def emit_scores(ki):
                        lead = max(ki * P - s0, 0)
                        sp = stvt.tile([P, ABLK], f32, tag="st", name="stps")
                        nc.tensor.matmul(
                            sp[:, lead:],
                            kT_blks[ki // 4][:, (ki % 4) * P : (ki % 4 + 1) * P],
                            qT[:, lead:],
                            start=True, stop=True,
                        )
                        st_ps_l[ki] = sp

                    def emit_exp(ki):
                        lead = max(ki * P - s0, 0)
                        stt = stsb.tile([P, ABLK], bf16, tag="stt", name="stt")
                        nc.scalar.activation(
                            stt[:, lead:], st_ps_l[ki][:, lead:], EXP,
                            scale=SCALE,
                        )
                        if ki * P >= s0:  # diagonal tile: zero above-diag
                            nc.vector.tensor_mul(
                                stt[:, lead : lead + P],
                                stt[:, lead : lead + P],
                                mask01_sb[:],
                            )
                        st_t_l[ki] = stt

                    def emit_pv(ki):
                        lead = max(ki * P - s0, 0)
                        first = ki == 0
                        last = ki == n_sk - 1
                        nc.tensor.matmul(
                            oT_ps[:, lead:],
                            v_blks[ki // 4][:, ki % 4, :],
                            st_t_l[ki][:, lead:],
                            start=first, stop=last,
                        )
                        nc.tensor.matmul(
                            sm_ps[:, lead:],
                            ones_sb[:],
                            st_t_l[ki][:, lead:],
                            start=first, stop=last,
                        )
                        st_t_l[ki] = None
                        st_ps_l[ki] = None

                    # software-pipelined emission, skew 2
                    for ki in range(n_sk):
                        emit_scores(ki)
                        if ki >= 1:
                            emit_exp(ki - 1)
                        if ki >= 2:
                            emit_pv(ki - 2)
                    emit_exp(n_sk - 1)
                    if n_sk >= 2:
                        emit_pv(n_sk - 2)
                    emit_pv(n_sk - 1)

                    rc = nrm.tile([P, ABLK], f32, tag="rc", name="rc")
                    nc.vector.reciprocal(rc[:], sm_ps[:])
                    nc.vector.tensor_mul(
                        oT_blks[blk][:, h, :], oT_ps[:], rc[:])

            def outproj(blk):
                for t in range(ABLK // P):
                    row = blk * (ABLK // P) + t
                    for cb in range(D // 512):
                        op_ps = opl.tile([P, 512], f32, tag="op", name="opps")
                        for h in range(QPH):
                            nc.tensor.matmul(
                                op_ps[:],
                                oT_blks[blk][:, h, t * P : (t + 1) * P],
                                wo_sb[:, h, cb * 512 : (cb + 1) * 512],
                                start=(h == 0), stop=(h == QPH - 1),
                            )
                        ob = osg.tile([P, 512], f32, tag="ob", name="ob")
                        nc.vector.tensor_scalar_mul(ob[:], op_ps[:], 1.0)
                        nc.sync.dma_start(
                            out=outp[row * P : (row + 1) * P,
                                     cb * 512 : (cb + 1) * 512],
                            in_=ob[:],
                        )

            for blk in range(NA):
                proj(blk)
                attn(blk)
                outproj(blk)

    _strip_pe_self_waits(nc)
    nc.finalize()
    return nc


def _strip_pe_self_waits(nc):
    """Remove PE-on-PE semaphore waits from PE matmuls.

    Tile's semaphore assigner emits conservative same-proc waits for
    PSUM-bank WAW reuse.  They are always satisfied by program order (PE
    matmuls complete strictly in order, and ldweights pull-ahead only
    reads SBUF, which PE never writes), and stripping them frees the
    single sync-wait slot of self-loading matmul forms for the real
    cross-engine dependency.
    """
    import concourse.mybir as mybir

    stripped = 0
    for bb in nc.m.functions[0].blocks:
        for inst in bb.instructions:
            si = getattr(inst, "sync_info", None)
            if si is None or not getattr(si, "on_wait", None):
                continue
            if isinstance(inst, mybir.InstMatmult):
                keep = [
                    w for w in si.on_wait
                    if not (w.sync_type == "semaphore"
                            and w.ant_name.startswith("PE"))
                ]
                stripped += len(si.on_wait) - len(keep)
                si.on_wait = keep
    return stripped


def _bf16(a):
    import ml_dtypes

    return np.asarray(a, np.float32).astype(ml_dtypes.bfloat16)


def _prep_inputs(x, freqs_cos, freqs_sin, Wq, Wk, Wv, Wo):
    """Build the 8 per-core input maps (pure layout work, no arithmetic)."""
    perm = np.concatenate([np.arange(0, HD, 2), np.arange(1, HD, 2)])

    cosT = freqs_cos.T.astype(np.float32)  # [64, S]
    sinT = freqs_sin.T.astype(np.float32)
    cc = _bf16(np.concatenate([cosT, cosT], axis=0))          # [128, S]
    sspm = _bf16(np.concatenate([sinT, -sinT], axis=0))       # [128, S]
    ones = _bf16(np.ones((P, P), np.float32))
    ident = _bf16(np.eye(P, dtype=np.float32))
    # mask01[p, j] = 1 where j >= p (keep), else 0
    mask01 = _bf16(np.triu(np.ones((P, P), np.float32)))

    # xtb[blk, p, di, s] = x[b].T[di*128+p, blk*512+s]
    xtbs = []
    for b in range(B):
        xT = np.ascontiguousarray(x[b].T.astype(np.float32))  # [D, S]
        t = xT.reshape(ND, P, NA, ABLK).transpose(2, 1, 0, 3)
        xtbs.append(_bf16(np.ascontiguousarray(t)))

    wqs, wks, wvs, wos = [], [], [], []
    for g in range(G):
        wq_g = Wq[:, g * EQ : (g + 1) * EQ].reshape(D, QPH, HD)[:, :, perm]
        wq_g = wq_g.reshape(D, EQ).reshape(ND, P, EQ).transpose(1, 0, 2)
        wqs.append(_bf16(np.ascontiguousarray(wq_g)))         # [128, 16, 512]
        wk_g = Wk[:, g * HD : (g + 1) * HD][:, perm]
        wk_g = wk_g.reshape(ND, P, HD).transpose(1, 0, 2)
        wks.append(_bf16(np.ascontiguousarray(wk_g)))         # [128, 16, 128]
        wv_g = Wv[:, g * HD : (g + 1) * HD]
        wv_g = wv_g.reshape(ND, P, HD).transpose(1, 0, 2)
        wvs.append(_bf16(np.ascontiguousarray(wv_g)))
        wo_g = Wo[g * EQ : (g + 1) * EQ, :]                   # [512, D]
        wo_g = wo_g.reshape(QPH, P, D).transpose(1, 0, 2)
        wos.append(_bf16(np.ascontiguousarray(wo_g)))         # [128, 4, 2048]

    in_maps = []
    for c in range(NCORES):
        b, g = divmod(c, G)
        in_maps.append(
            dict(xtb=xtbs[b], wq=wqs[g], wk=wks[g], wv=wvs[g], wo=wos[g],
                 cc=cc, sspm=sspm, ones_d=ones, ident_d=ident,
                 mask01_d=mask01)
        )
    return in_maps


LAST_RESULTS = None


def kernel(**inputs) -> np.ndarray:
    global LAST_RESULTS
    x = np.asarray(inputs["x"], np.float32)
    in_maps = _prep_inputs(
        x,
        np.asarray(inputs["freqs_cos"], np.float32),
        np.asarray(inputs["freqs_sin"], np.float32),
        np.asarray(inputs["Wq"], np.float32),
        np.asarray(inputs["Wk"], np.float32),
        np.asarray(inputs["Wv"], np.float32),
        np.asarray(inputs["Wo"], np.float32),
    )

    if "nc" not in _CACHE:
        _CACHE["nc"] = _build_program()
    nc = _CACHE["nc"]

    from concourse import bass_utils

    res = bass_utils.run_bass_kernel_spmd(nc, in_maps, list(range(NCORES)))
    LAST_RESULTS = res

    out = np.empty((B, S, D), np.float32)
    for b in range(B):
        acc = res.results[4 * b]["outp"].astype(np.float32)
        for g in range(1, G):
            acc = acc + res.results[4 * b + g]["outp"]
        out[b] = acc
    return out
